# revision 1
# baseline (speedup 1.0000x reference)
"""Trainium2 Bass kernel for the masked block-diagonal LSTM net.

Model structure (hardcoded from the problem spec):
  - x_seq [512, 64, 32], recurrent state HID=1088 = 34 blocks x 32.
  - U projections are masked so hidden block j only sees input feature j
    (block 0 additionally sees features 0,1 again via the interaction rows);
    hidden blocks 32,33 receive NO input projection at all.
  - V recurrent matrices are masked block-diagonal -> the 34 blocks evolve
    completely independently through the scan.

Sharding: hidden-block parallel. Cores 0..7 each own 4 input-driven blocks
(128 hidden rows) x the full batch 512. Layout on device is h^T:
[hid on partitions, batch on free dim], so the recurrent matmul, the gate
activations and the state updates all run at full 128-partition width with
N=512 columns and no transposes anywhere.

Blocks 32,33 are bias-only (no x dependence): their state is identical for
every batch element, so their scalar contribution to the readout (and the
tiny 16-feature static MLP + final sigmoid) is folded into the host-side
unshard step.
"""

import sys

sys.path.insert(0, "/opt/trn_rl_repo")

import numpy as np

B = 512
T = 64
INPUT_SZ = 32
HPF = 32
INTER = [(0, 1), (2, 3)]
NB = INPUT_SZ + len(INTER)  # 34
HID = NB * HPF  # 1088
IN_SZ = INPUT_SZ + 2 * len(INTER)  # 36
F_STAT = 16
N_CORES = 8
BLOCKS_PER_CORE = 4
CORE_HID = BLOCKS_PER_CORE * HPF  # 128
CHUNKS = 2  # batch-column chunks per step (pipelining granularity)
CB = B // CHUNKS

_CACHE = {}


def _build_masks():
    um = np.zeros((IN_SZ, HID), np.float32)
    for i in range(INPUT_SZ):
        um[i, i * HPF : (i + 1) * HPF] = 1.0
    for i in range(0, len(INTER), 2):
        um[i + INPUT_SZ, i * HPF : (i + 1) * HPF] = 1.0
        um[i + INPUT_SZ + 1, i * HPF : (i + 1) * HPF] = 1.0
    vm = np.kron(np.eye(NB, dtype=np.float32), np.ones((HPF, HPF), np.float32))
    return um, vm


def _build_program(repeat=1, loop_n=0):
    # repeat>1 duplicates the whole computation serially (same I/O).
    # loop_n>0 instead wraps ONE copy in a hardware For_i loop executing
    # loop_n times: program size stays constant, so wall-clock deltas
    # between two loop_n values isolate true device execution time from
    # the per-call NEFF dispatch overhead (which scales with program size).
    import concourse.bass as bass
    import concourse.tile as tile
    from concourse import bacc, mybir
    from contextlib import nullcontext

    f32 = mybir.dt.float32
    f16 = mybir.dt.float16
    ACT = mybir.ActivationFunctionType

    nc = bacc.Bacc("TRN2", target_bir_lowering=False, debug=False)

    xf_d = nc.dram_tensor("xf", [5, T * B], f16, kind="ExternalInput").ap()
    wu_d = nc.dram_tensor("wu", [4, 5, CORE_HID], f16, kind="ExternalInput").ap()
    wv_d = nc.dram_tensor("wv", [4, CORE_HID, CORE_HID], f16, kind="ExternalInput").ap()
    oc_d = nc.dram_tensor("oc", [CORE_HID, 1], f16, kind="ExternalInput").ap()
    part_d = nc.dram_tensor("partial", [1, B], f32, kind="ExternalOutput").ap()

    with tile.TileContext(nc) as tc:
        with (
            tc.tile_pool(name="const", bufs=1) as cpool,
            tc.tile_pool(name="state", bufs=2) as spool,
            tc.tile_pool(name="work", bufs=3) as wpool,
            tc.tile_pool(name="psum", bufs=2, space="PSUM") as ppool,
        ):
            xf = cpool.tile([5, T * B], f16, tag="xf")
            nc.sync.dma_start(xf[:], xf_d[:])
            wu = []
            wv = []
            for g in range(4):
                wut = cpool.tile([5, CORE_HID], f16, tag=f"wu{g}")
                nc.sync.dma_start(wut[:], wu_d[g])
                wu.append(wut)
                wvt = cpool.tile([CORE_HID, CORE_HID], f16, tag=f"wv{g}")
                nc.sync.dma_start(wvt[:], wv_d[g])
                wv.append(wvt)
            oc = cpool.tile([CORE_HID, 1], f16, tag="oc")
            nc.sync.dma_start(oc[:], oc_d[:])

            loop_cm = (lambda: tc.For_i(0, loop_n, 1)) if loop_n else None
            for rep in range(repeat):
              with loop_cm() if loop_cm else nullcontext():
                # per-chunk state tiles -> exact dependency granularity
                hs_t = []
                cs_t = []
                for ch in range(CHUNKS):
                    h0 = spool.tile([CORE_HID, CB], f16, tag=f"h{ch}")
                    c0 = spool.tile([CORE_HID, CB], f32, tag=f"c{ch}")
                    nc.vector.memset(h0[:].bitcast(mybir.dt.uint16), 0)
                    nc.vector.memset(c0[:], 0.0)
                    hs_t.append(h0)
                    cs_t.append(c0)

                for t in range(T):
                    for ch in range(CHUNKS):
                        h, c = hs_t[ch], cs_t[ch]
                        # own psum tile per chunk: [128, 4 gates, CB]
                        ps = ppool.tile([128, 4, CB], f32, tag=f"ps{ch}")
                        for g in (0, 1, 2, 3):
                            out = ps[:, g]
                            nc.tensor.matmul(
                                out,
                                wu[g][:],
                                xf[:, t * B + ch * CB : t * B + (ch + 1) * CB],
                                start=True,
                                stop=False,
                            )
                            nc.tensor.matmul(
                                out, wv[g][:], h[:], start=False, stop=True
                            )
                        # one fused sigmoid over all 4 gate banks; the cell
                        # gate's weights are pre-scaled x2 so bank 3 yields
                        # g' = sigmoid(2y) with tanh(y) = 2g' - 1
                        ifog = wpool.tile([CORE_HID, 4, CB], f16, tag=f"ifog{ch}")
                        nc.scalar.activation(ifog[:], ps[:], ACT.Sigmoid)
                        i_, f_, o_, g_ = (ifog[:, k] for k in range(4))
                        # c_new = f*c + (2*(i*g') - i)
                        t1 = wpool.tile([CORE_HID, CB], f16, tag=f"t1{ch}")
                        nc.gpsimd.tensor_mul(t1[:], f_, c[:])  # f*c on Pool
                        t2 = wpool.tile([CORE_HID, CB], f16, tag=f"t2{ch}")
                        nc.vector.tensor_mul(t2[:], i_, g_)  # i*g'
                        u = wpool.tile([CORE_HID, CB], f16, tag=f"u{ch}")
                        nc.vector.scalar_tensor_tensor(
                            u[:], t2[:], 2.0, i_, mybir.AluOpType.mult,
                            mybir.AluOpType.subtract,
                        )
                        c_new = spool.tile([CORE_HID, CB], f16, tag=f"c{ch}")
                        nc.vector.tensor_add(c_new[:], t1[:], u[:])
                        # tanh(c) = 2*sigmoid(2c) - 1
                        sc = wpool.tile([CORE_HID, CB], f16, tag=f"sc{ch}")
                        nc.scalar.activation(sc[:], c_new[:], ACT.Sigmoid, scale=2.0)
                        t3 = wpool.tile([CORE_HID, CB], f16, tag=f"t3{ch}")
                        nc.vector.tensor_mul(t3[:], o_, sc[:])  # o*sc
                        h_new = spool.tile([CORE_HID, CB], f16, tag=f"h{ch}")
                        nc.vector.scalar_tensor_tensor(
                            h_new[:], t3[:], 2.0, o_, mybir.AluOpType.mult,
                            mybir.AluOpType.subtract,
                        )
                        hs_t[ch] = h_new
                        cs_t[ch] = c_new

                # readout partial: oc^T @ h  -> [1, B]
                outsb = wpool.tile([1, B], f32, tag="outsb")
                for ch in range(CHUNKS):
                    pr = ppool.tile([128, 4, CB], f32, tag=f"ps{ch}")
                    nc.tensor.matmul(
                        pr[0:1, 0], oc[:], hs_t[ch][:], start=True, stop=True
                    )
                    nc.vector.tensor_copy(outsb[:, ch * CB : (ch + 1) * CB], pr[0:1, 0])
                nc.sync.dma_start(part_d[:], outsb[:])

    nc.compile()
    return nc


def _pack_inputs(inputs):
    um, vm = _build_masks()
    gates = [
        (inputs["U_i"], inputs["V_i"], inputs["b_i"]),
        (inputs["U_f"], inputs["V_f"], inputs["b_f"]),
        (inputs["U_o"], inputs["V_o"], inputs["b_o"]),
        (inputs["U_c"], inputs["V_c"], inputs["b_c"]),
    ]
    Up = [np.asarray(U, np.float32) * um for U, _, _ in gates]
    Vp = [np.asarray(V, np.float32) * vm for _, V, _ in gates]
    bs = [np.asarray(b, np.float32) for _, _, b in gates]
    x_seq = np.asarray(inputs["x_seq"], np.float32)
    out_coef = np.asarray(inputs["out_coef"], np.float32)

    in_maps = []
    for core in range(N_CORES):
        feats = list(range(4 * core, 4 * core + 4))
        hs = slice(CORE_HID * core, CORE_HID * (core + 1))
        xf = np.ones((5, T * B), np.float32)
        # column index = t*B + b
        xf[0:4] = x_seq[:, :, feats].transpose(2, 1, 0).reshape(4, T * B)
        wu = np.zeros((4, 5, CORE_HID), np.float32)
        wv = np.zeros((4, CORE_HID, CORE_HID), np.float32)
        for g in range(4):
            wu[g, 0:4] = Up[g][feats, hs]
            if core == 0:
                # interaction rows 32,33 multiply x0,x1 -> fold into rows 0,1
                wu[g, 0] += Up[g][32, hs]
                wu[g, 1] += Up[g][33, hs]
            wu[g, 4] = bs[g][hs]
            wv[g] = Vp[g][hs, hs]
        # cell gate (idx 3) pre-scaled x2: tanh(y) = 2*sigmoid(2y) - 1
        wu[3] *= 2.0
        wv[3] *= 2.0
        in_maps.append(
            {
                "xf": xf.astype(np.float16),
                "wu": wu.astype(np.float16),
                "wv": wv.astype(np.float16),
                "oc": np.ascontiguousarray(out_coef[hs]).astype(np.float16),
            }
        )
    return in_maps, Vp, bs, out_coef


def _host_tail(inputs, partials, Vp, bs, out_coef):
    """Bias-only blocks 32,33 (batch-independent scalar) + static MLP +
    final sigmoid. All exact model math, done during unshard."""
    aux = slice(32 * HPF, HID)  # hid 1024:1088
    h = np.zeros(2 * HPF, np.float32)
    cst = np.zeros(2 * HPF, np.float32)
    Va = [V[aux, aux] for V in Vp]
    ba = [b[aux] for b in bs]

    def sig(x):
        return 1.0 / (1.0 + np.exp(-x))

    for _ in range(T):
        i_t = sig(ba[0] + h @ Va[0])
        f_t = sig(ba[1] + h @ Va[1])
        o_t = sig(ba[2] + h @ Va[2])
        g_t = np.tanh(ba[3] + h @ Va[3])
        cst = f_t * cst + i_t * g_t
        h = o_t * np.tanh(cst)
    s_aux = float(h @ out_coef[aux, 0])

    x_stat = np.asarray(inputs["x_stat"], np.float32)
    W1 = np.asarray(inputs["W1"], np.float32)
    b1 = np.asarray(inputs["b1"], np.float32)
    W2 = np.asarray(inputs["W2"], np.float32)
    b2 = np.asarray(inputs["b2"], np.float32)
    hid = np.maximum(x_stat[:, :, None] * W1[None] + b1[None], 0.0)
    mlp = sig(np.einsum("bfk,fk->bf", hid, W2) + b2)
    mlp_part = mlp @ out_coef[HID:, 0]

    z = partials.sum(axis=0) + s_aux + mlp_part + float(np.asarray(inputs["out_bias"])[0])
    return sig(z).astype(np.float32).reshape(B, 1)


def kernel(**inputs):
    from concourse.bass_utils import run_bass_kernel_spmd

    if "nc" not in _CACHE:
        _CACHE["nc"] = _build_program()
    nc = _CACHE["nc"]

    in_maps, Vp, bs, out_coef = _pack_inputs(inputs)
    res = run_bass_kernel_spmd(nc, in_maps, core_ids=list(range(N_CORES)))
    partials = np.stack([res.results[c]["partial"][0] for c in range(N_CORES)])
    return _host_tail(inputs, partials, Vp, bs, out_coef)



# revision 3
# speedup vs baseline: 1.7159x; 1.7159x over previous
"""Trainium2 Bass kernel for the masked block-diagonal LSTM net.

Model structure (hardcoded from the problem spec):
  - x_seq [512, 64, 32], recurrent state HID=1088 = 34 blocks x 32.
  - U projections are masked so hidden block j only sees input feature j
    (block 0 additionally sees features 0,1 again via the interaction rows);
    hidden blocks 32,33 receive NO input projection at all.
  - V recurrent matrices are masked block-diagonal -> the 34 blocks evolve
    completely independently through the scan.

Sharding: hidden-block parallel. Cores 0..7 each own 4 input-driven blocks
(128 hidden rows) x the full batch 512. Layout on device is h^T:
[hid on partitions, batch on free dim], so the recurrent matmul, the gate
activations and the state updates all run at full 128-partition width with
N=512 columns and no transposes anywhere.

Blocks 32,33 are bias-only (no x dependence): their state is identical for
every batch element, so their scalar contribution to the readout (and the
tiny 16-feature static MLP + final sigmoid) is folded into the host-side
unshard step.

v2 restructure (vs the first working version):
  - t=0 recurrent matmuls skipped entirely (h0 == 0).
  - input-projection matmuls for step t+1 are emitted during step t and
    paired per gate across the two batch chunks so the PE can reuse the
    loaded weights and stay busy while the elementwise chain runs.
  - scalar_tensor_tensor ops (no DVE perf modes, ~594ns measured) replaced
    with tensor_scalar immediate ops (4x mode) and a real Tanh activation:
      cell gate: g2 = 2*sigmoid(2y) - 1 == tanh(y)   (weights pre-scaled x2)
      h update:  h  = o * tanh(c)                     (direct Tanh act)
"""

import sys

sys.path.insert(0, "/opt/trn_rl_repo")

import numpy as np

B = 512
T = 64
INPUT_SZ = 32
HPF = 32
INTER = [(0, 1), (2, 3)]
NB = INPUT_SZ + len(INTER)  # 34
HID = NB * HPF  # 1088
IN_SZ = INPUT_SZ + 2 * len(INTER)  # 36
F_STAT = 16
N_CORES = 8
BLOCKS_PER_CORE = 4
CORE_HID = BLOCKS_PER_CORE * HPF  # 128
CHUNKS = 2  # batch-column chunks per step (pipelining granularity)
CB = B // CHUNKS

_CACHE = {}


def _build_masks():
    um = np.zeros((IN_SZ, HID), np.float32)
    for i in range(INPUT_SZ):
        um[i, i * HPF : (i + 1) * HPF] = 1.0
    for i in range(0, len(INTER), 2):
        um[i + INPUT_SZ, i * HPF : (i + 1) * HPF] = 1.0
        um[i + INPUT_SZ + 1, i * HPF : (i + 1) * HPF] = 1.0
    vm = np.kron(np.eye(NB, dtype=np.float32), np.ones((HPF, HPF), np.float32))
    return um, vm


def _build_program(repeat=1, loop_n=0):
    # repeat>1 duplicates the whole computation serially (same I/O).
    # loop_n>0 instead wraps ONE copy in a hardware For_i loop executing
    # loop_n times: program size stays constant, so wall-clock deltas
    # between two loop_n values isolate true device execution time from
    # the per-call NEFF dispatch overhead (which scales with program size).
    import concourse.bass as bass
    import concourse.tile as tile
    from concourse import bacc, mybir
    from contextlib import nullcontext

    f32 = mybir.dt.float32
    f16 = mybir.dt.float16
    ACT = mybir.ActivationFunctionType
    ALU = mybir.AluOpType

    nc = bacc.Bacc("TRN2", target_bir_lowering=False, debug=False)

    xf_d = nc.dram_tensor("xf", [5, T * B], f16, kind="ExternalInput").ap()
    wu_d = nc.dram_tensor("wu", [4, 5, CORE_HID], f16, kind="ExternalInput").ap()
    wv_d = nc.dram_tensor("wv", [4, CORE_HID, CORE_HID], f16, kind="ExternalInput").ap()
    oc_d = nc.dram_tensor("oc", [CORE_HID, 1], f16, kind="ExternalInput").ap()
    part_d = nc.dram_tensor("partial", [1, B], f32, kind="ExternalOutput").ap()

    with tile.TileContext(nc) as tc:
        with (
            tc.tile_pool(name="const", bufs=1) as cpool,
            tc.tile_pool(name="state", bufs=2) as spool,
            tc.tile_pool(name="work", bufs=3) as wpool,
            tc.tile_pool(name="psum", bufs=2, space="PSUM") as ppool,
        ):
            xf = cpool.tile([5, T * B], f16, tag="xf")
            nc.sync.dma_start(xf[:], xf_d[:])
            wu = []
            wv = []
            for g in range(4):
                wut = cpool.tile([5, CORE_HID], f16, tag=f"wu{g}")
                nc.sync.dma_start(wut[:], wu_d[g])
                wu.append(wut)
                wvt = cpool.tile([CORE_HID, CORE_HID], f16, tag=f"wv{g}")
                nc.sync.dma_start(wvt[:], wv_d[g])
                wv.append(wvt)
            oc = cpool.tile([CORE_HID, 1], f16, tag="oc")
            nc.sync.dma_start(oc[:], oc_d[:])

            def emit_inp(ps_tiles, t, stop):
                # input projections for step t, paired per gate across
                # chunks so the PE can reuse the loaded weights
                for g in range(4):
                    for ch in range(CHUNKS):
                        nc.tensor.matmul(
                            ps_tiles[ch][:, g],
                            wu[g][:],
                            xf[:, t * B + ch * CB : t * B + (ch + 1) * CB],
                            start=True,
                            stop=stop,
                        )

            loop_cm = (lambda: tc.For_i(0, loop_n, 1)) if loop_n else None
            for rep in range(repeat):
              with loop_cm() if loop_cm else nullcontext():
                cs_t = []
                hs_t = [None, None]
                for ch in range(CHUNKS):
                    c0 = spool.tile([CORE_HID, CB], f16, tag=f"c{ch}")
                    nc.vector.memset(c0[:].bitcast(mybir.dt.uint16), 0)
                    cs_t.append(c0)

                ps_cur = [
                    ppool.tile([128, 4, CB], f32, tag=f"ps{ch}", name=f"ps{ch}")
                    for ch in range(CHUNKS)
                ]
                emit_inp(ps_cur, 0, stop=True)

                for t in range(T):
                    ifogs = []
                    for ch in range(CHUNKS):
                        if t > 0:
                            for g in range(4):
                                nc.tensor.matmul(
                                    ps_cur[ch][:, g],
                                    wv[g][:],
                                    hs_t[ch][:],
                                    start=False,
                                    stop=True,
                                )
                        # one fused sigmoid over all 4 gate banks; the cell
                        # gate's weights are pre-scaled x2 so bank 3 yields
                        # g' = sigmoid(2y) with tanh(y) = 2g' - 1
                        ifog = wpool.tile([CORE_HID, 4, CB], f16, tag=f"ifog{ch}")
                        nc.scalar.activation(ifog[:], ps_cur[ch][:], ACT.Sigmoid)
                        ifogs.append(ifog)

                    if t + 1 < T:
                        ps_next = [
                            ppool.tile([128, 4, CB], f32, tag=f"ps{ch}", name=f"ps{ch}")
                            for ch in range(CHUNKS)
                        ]
                        emit_inp(ps_next, t + 1, stop=False)
                    else:
                        ps_next = None

                    for ch in range(CHUNKS):
                        ifog = ifogs[ch]
                        i_, f_, o_, g_ = (ifog[:, k] for k in range(4))
                        # tanh(y) = 2*sigmoid(2y) - 1 (4x-mode tensor_scalar)
                        g2 = wpool.tile([CORE_HID, CB], f16, tag=f"g2{ch}")
                        nc.vector.tensor_scalar(
                            g2[:], g_, 2.0, -1.0, ALU.mult, ALU.add
                        )
                        t2 = wpool.tile([CORE_HID, CB], f16, tag=f"t2{ch}")
                        nc.vector.tensor_mul(t2[:], i_, g2[:])  # i*tanh(y)
                        t1 = wpool.tile([CORE_HID, CB], f16, tag=f"t1{ch}")
                        nc.vector.tensor_mul(t1[:], f_, cs_t[ch][:])  # f*c
                        c_new = spool.tile([CORE_HID, CB], f16, tag=f"c{ch}")
                        nc.vector.tensor_add(c_new[:], t1[:], t2[:])
                        sc = wpool.tile([CORE_HID, CB], f16, tag=f"sc{ch}")
                        nc.scalar.activation(sc[:], c_new[:], ACT.Tanh)
                        h_new = spool.tile([CORE_HID, CB], f16, tag=f"h{ch}")
                        nc.vector.tensor_mul(h_new[:], o_, sc[:])
                        hs_t[ch] = h_new
                        cs_t[ch] = c_new

                    ps_cur = ps_next

                # readout partial: oc^T @ h  -> [1, B]
                outsb = wpool.tile([1, B], f32, tag="outsb")
                for ch in range(CHUNKS):
                    pr = ppool.tile([128, 4, CB], f32, tag=f"ps{ch}")
                    nc.tensor.matmul(
                        pr[0:1, 0], oc[:], hs_t[ch][:], start=True, stop=True
                    )
                    nc.vector.tensor_copy(outsb[:, ch * CB : (ch + 1) * CB], pr[0:1, 0])
                nc.sync.dma_start(part_d[:], outsb[:])

    nc.compile()
    return nc


def _pack_inputs(inputs):
    um, vm = _build_masks()
    gates = [
        (inputs["U_i"], inputs["V_i"], inputs["b_i"]),
        (inputs["U_f"], inputs["V_f"], inputs["b_f"]),
        (inputs["U_o"], inputs["V_o"], inputs["b_o"]),
        (inputs["U_c"], inputs["V_c"], inputs["b_c"]),
    ]
    Up = [np.asarray(U, np.float32) * um for U, _, _ in gates]
    Vp = [np.asarray(V, np.float32) * vm for _, V, _ in gates]
    bs = [np.asarray(b, np.float32) for _, _, b in gates]
    x_seq = np.asarray(inputs["x_seq"], np.float32)
    out_coef = np.asarray(inputs["out_coef"], np.float32)

    in_maps = []
    for core in range(N_CORES):
        feats = list(range(4 * core, 4 * core + 4))
        hs = slice(CORE_HID * core, CORE_HID * (core + 1))
        xf = np.ones((5, T * B), np.float32)
        # column index = t*B + b
        xf[0:4] = x_seq[:, :, feats].transpose(2, 1, 0).reshape(4, T * B)
        wu = np.zeros((4, 5, CORE_HID), np.float32)
        wv = np.zeros((4, CORE_HID, CORE_HID), np.float32)
        for g in range(4):
            wu[g, 0:4] = Up[g][feats, hs]
            if core == 0:
                # interaction rows 32,33 multiply x0,x1 -> fold into rows 0,1
                wu[g, 0] += Up[g][32, hs]
                wu[g, 1] += Up[g][33, hs]
            wu[g, 4] = bs[g][hs]
            wv[g] = Vp[g][hs, hs]
        # cell gate (idx 3) pre-scaled x2: tanh(y) = 2*sigmoid(2y) - 1
        wu[3] *= 2.0
        wv[3] *= 2.0
        in_maps.append(
            {
                "xf": xf.astype(np.float16),
                "wu": wu.astype(np.float16),
                "wv": wv.astype(np.float16),
                "oc": np.ascontiguousarray(out_coef[hs]).astype(np.float16),
            }
        )
    return in_maps, Vp, bs, out_coef


def _host_tail(inputs, partials, Vp, bs, out_coef):
    """Bias-only blocks 32,33 (batch-independent scalar) + static MLP +
    final sigmoid. All exact model math, done during unshard."""
    aux = slice(32 * HPF, HID)  # hid 1024:1088
    h = np.zeros(2 * HPF, np.float32)
    cst = np.zeros(2 * HPF, np.float32)
    Va = [V[aux, aux] for V in Vp]
    ba = [b[aux] for b in bs]

    def sig(x):
        return 1.0 / (1.0 + np.exp(-x))

    for _ in range(T):
        i_t = sig(ba[0] + h @ Va[0])
        f_t = sig(ba[1] + h @ Va[1])
        o_t = sig(ba[2] + h @ Va[2])
        g_t = np.tanh(ba[3] + h @ Va[3])
        cst = f_t * cst + i_t * g_t
        h = o_t * np.tanh(cst)
    s_aux = float(h @ out_coef[aux, 0])

    x_stat = np.asarray(inputs["x_stat"], np.float32)
    W1 = np.asarray(inputs["W1"], np.float32)
    b1 = np.asarray(inputs["b1"], np.float32)
    W2 = np.asarray(inputs["W2"], np.float32)
    b2 = np.asarray(inputs["b2"], np.float32)
    hid = np.maximum(x_stat[:, :, None] * W1[None] + b1[None], 0.0)
    mlp = sig(np.einsum("bfk,fk->bf", hid, W2) + b2)
    mlp_part = mlp @ out_coef[HID:, 0]

    z = partials.sum(axis=0) + s_aux + mlp_part + float(np.asarray(inputs["out_bias"])[0])
    return sig(z).astype(np.float32).reshape(B, 1)


def kernel(**inputs):
    from concourse.bass_utils import run_bass_kernel_spmd

    if "nc" not in _CACHE:
        _CACHE["nc"] = _build_program()
    nc = _CACHE["nc"]

    in_maps, Vp, bs, out_coef = _pack_inputs(inputs)
    res = run_bass_kernel_spmd(nc, in_maps, core_ids=list(range(N_CORES)))
    partials = np.stack([res.results[c]["partial"][0] for c in range(N_CORES)])
    return _host_tail(inputs, partials, Vp, bs, out_coef)
